# revision 22
# baseline (speedup 1.0000x reference)
"""Trainium2 Bass kernel for the vq_codebook problem.

Math (per batch row b, all computed on one NeuronCore; batch is data-parallel
across the 8 cores):
  xp[t, n]   = x[t] . proj_w[n]  (+ proj_b[n])          n = k*1024 + c
  x_loss[t,k]= logsumexp_c(xp[t,k,:]) - xp[t,k,tgt]     (no max-subtraction
                needed: |logits| <~ 7 so exp() is safe in fp32/fp16)
  xw[t,:]    = softmax_k(x[t] . wproj_w[k])
  emb[t,:]   = sum_k xw[t,k] * codebook[tgt[t,k], k, :]

Device layout choices:
  - tokens on partitions, codes on the free dim -> the softmax denominator
    falls out of the ScalarE exp pass via accum_out, and the CE "gather"
    is a one-hot tensor_scalar mask + tensor_tensor_reduce on VectorE.
  - codebook rows fetched with indirect DMA (gather) in (token,k)-pair
    partition order; the weighted sum over k is a tiny block-diagonal
    matmul producing emb transposed (d on partitions), which keeps the
    PSUM->SBUF copy at full 128-partition utilization.

Walrus wait-budget notes: several instruction encodings (TensorScalarPtr,
PSEUDO_DMA_DIRECT2D) accept only one sync-wait command, so small per-chunk
tensors are loaded once as resident constants, per-chunk DRAM bounce tiles
get bufs=CH (no slot reuse -> no WAR/WAW waits), and AP-scalar operands are
either produced on VectorE itself or replaced by broadcast tensor_tensor.
"""

import os
import sys

import numpy as np

if "/opt/trn_rl_repo" not in sys.path:
    sys.path.insert(0, "/opt/trn_rl_repo")

import ml_dtypes
from contextlib import ExitStack

import concourse.bass as bass
import concourse.tile as tile
from concourse import bacc
from concourse import mybir

F32 = mybir.dt.float32
BF16 = mybir.dt.bfloat16
F16 = mybir.dt.float16
I32 = mybir.dt.int32

D = 1024          # model dim
NK = 8            # num codebooks
NCODE = 1024      # codes per codebook
NG = NK * NCODE   # 8192 flat codes
N_CORES = 8

AF = mybir.ActivationFunctionType
OP = mybir.AluOpType


def build_bass(T: int = 1024):
    """Build the single-core Bass program (SPMD across cores)."""
    CH = T // 128          # token chunks of 128
    GR = 8                 # groups of 16 tokens per chunk
    nc = bacc.Bacc("TRN2", target_bir_lowering=False)

    xT = nc.dram_tensor("xT", [D, T], BF16, kind="ExternalInput")
    wT = nc.dram_tensor("wT", [D, NG], BF16, kind="ExternalInput")
    wpT = nc.dram_tensor("wpT", [D, NK], BF16, kind="ExternalInput")
    cbf = nc.dram_tensor("cbf", [NG, D], BF16, kind="ExternalInput")
    gidx = nc.dram_tensor("gidx", [128, T // 16], I32, kind="ExternalInput")
    tgtf = nc.dram_tensor("tgtf", [128, CH, NK], F32, kind="ExternalInput")
    m32 = nc.dram_tensor("m32", [128, 32], BF16, kind="ExternalInput")
    embT = nc.dram_tensor("embT", [D, T], BF16, kind="ExternalOutput")
    xl = nc.dram_tensor("xl", [T, NK], F32, kind="ExternalOutput")

    with tile.TileContext(nc) as tc, ExitStack() as ctx:
        const = ctx.enter_context(tc.tile_pool(name="const", bufs=1))
        small = ctx.enter_context(tc.tile_pool(name="small", bufs=2))
        epool = ctx.enter_context(tc.tile_pool(name="epool", bufs=2))
        spool = ctx.enter_context(tc.tile_pool(name="spool", bufs=2))
        gpool = ctx.enter_context(tc.tile_pool(name="gpool", bufs=3))
        opool = ctx.enter_context(tc.tile_pool(name="opool", bufs=2))
        nobuf = ctx.enter_context(tc.tile_pool(name="nobuf", bufs=CH))
        drm = ctx.enter_context(tc.tile_pool(name="drm", bufs=CH, space="DRAM"))
        psl = ctx.enter_context(tc.tile_pool(name="psl", bufs=2, space="PSUM"))
        psw = ctx.enter_context(tc.tile_pool(name="psw", bufs=1, space="PSUM"))
        pse = ctx.enter_context(tc.tile_pool(name="pse", bufs=1, space="PSUM"))

        # resident inputs. Small/early tensors go first on the SWDGE path
        # (async descriptor generation); the 16MB weight load is sliced and
        # alternated across the two DMA paths so no single serial ring
        # head-of-line-blocks the tensors the first matmuls need.
        tg_al0 = const.tile([128, CH, NK], F32)
        nc.gpsimd.dma_start(tg_al0[:], tgtf[:])
        tg_all = const.tile([128, CH, NK], F32)
        nc.vector.tensor_copy(tg_all[:], tg_al0[:])
        git_all = const.tile([128, T // 16], I32)
        nc.gpsimd.dma_start(git_all[:], gidx[:])
        m32_t = const.tile([128, 32], BF16)
        nc.gpsimd.dma_start(m32_t[:], m32[:])
        m32c = const.tile([128, 32], BF16)
        nc.vector.tensor_copy(m32c[:], m32_t[:])
        wp_t = const.tile([128, 8, NK], BF16)
        nc.gpsimd.dma_start(wp_t[:], wpT.rearrange("(dc p) k -> p dc k", p=128))
        x_t = const.tile([128, 8, T], BF16)
        xTr = xT.rearrange("(dc p) t -> p dc t", p=128)
        for dc in range(8):
            nc.gpsimd.dma_start(x_t[:, dc, :], xTr[:, dc, :])
        w_t = const.tile([128, 8, NG], BF16)
        wTr = wT.rearrange("(dc p) n -> p dc n", p=128)
        for k in range(NK):
            eng = nc.sync if k % 2 == 0 else nc.gpsimd
            ksl = slice(k * NCODE, (k + 1) * NCODE)
            eng.dma_start(w_t[:, :, ksl], wTr[:, :, ksl])
        # iota 0..NCODE-1 per partition; the fp16 copy is DVE-produced so the
        # mask TensorScalarPtr carries no extra semaphore waits.
        iota_i = const.tile([128, NCODE], mybir.dt.int16)
        nc.gpsimd.iota(iota_i[:], pattern=[[1, NCODE]], base=0, channel_multiplier=0)
        iota_t = const.tile([128, NCODE], F16)
        nc.vector.tensor_copy(iota_t[:], iota_i[:])

        xwg_list = []
        S_list = [nobuf.tile([128, NK], F32, tag=f"S{c}", name=f"S{c}") for c in range(CH)]
        G_list = [nobuf.tile([128, NK], F32, tag=f"G{c}", name=f"G{c}") for c in range(CH)]

        for k in range(NK):
            for ch in range(CH):
                tsl = slice(ch * 128, (ch + 1) * 128)
                ps = psl.tile([128, NCODE], F32, tag="lg")
                if k == 0:
                    pw = psw.tile([128, NK], F32, tag="pw")
                for d in range(8):
                    lhs = x_t[:, d, tsl]
                    for nb in range(2):
                        nc.tensor.matmul(
                            ps[:, nb * 512 : (nb + 1) * 512],
                            lhs,
                            w_t[:, d, k * NCODE + nb * 512 : k * NCODE + (nb + 1) * 512],
                            start=(d == 0),
                            stop=(d == 7),
                        )
                    if k == 0:
                        nc.tensor.matmul(
                            pw[:], lhs, wp_t[:, d, :], start=(d == 0), stop=(d == 7)
                        )
                E = epool.tile([128, NCODE], F16, tag="E")
                nc.scalar.activation(
                    E[:], ps[:], AF.Exp, accum_out=S_list[ch][:, k : k + 1]
                )
                # G[t] = sum_c (iota_c == tgt_t) * E[t,c] = E[t, tgt_t]
                scrap = spool.tile([128, NCODE], F16, tag="scrap")
                nc.vector.scalar_tensor_tensor(
                    out=scrap[:],
                    in0=iota_t[:],
                    scalar=tg_all[:, ch, k : k + 1],
                    in1=E[:],
                    op0=OP.is_equal,
                    op1=OP.mult,
                    accum_out=G_list[ch][:, k : k + 1],
                )

                # xw softmax for this chunk (needs pw, k==0 only)
                if k == 0:
                    Ew = small.tile([128, NK], F32, tag="Ew")
                    Sw = small.tile([128, 1], F32, tag="Sw")
                    nc.scalar.activation(Ew[:], pw[:], AF.Exp, accum_out=Sw[:])
                    Rw = small.tile([128, 1], F32, tag="Rw")
                    nc.vector.reciprocal(Rw[:], Sw[:])
                    xwb = nobuf.tile([128, NK], BF16, tag="xwb")
                    nc.vector.tensor_tensor(
                        xwb[:], Ew[:], Rw[:, 0:1].to_broadcast([128, NK]), op=OP.mult
                    )
                    # repack xw from (t, k) to partition order p=(t%16)*8+k via
                    # DRAM round-trip: row-major (t,k) == ((g,m),k) == (g,(m,k))
                    xwd_d = drm.tile([128, NK], BF16, tag="xwdram")
                    nc.gpsimd.dma_start(xwd_d[:], xwb[:])
                    xwg = nobuf.tile([128, GR], BF16, tag="xwg", name=f"xwg{ch}")
                    nc.gpsimd.dma_start(
                        xwg[:], xwd_d[:].rearrange("(g m) k -> (m k) g", g=GR)
                    )
                    xwg_list.append(xwg)

                # emb for chunk ch, spread across k-iterations 1..7 so the
                # gather DMAs don't pile up behind one k-pass
                if k != 1 + (ch % 7):
                    continue
                xwg = xwg_list[ch]
                pe_t = pse.tile([128, 8, 128], F32, tag="pe")
                for g in range(GR):
                    xwdg = small.tile([128, 16], BF16, tag="xwdg")
                    nc.vector.tensor_tensor(
                        xwdg[:],
                        m32c[:, 0:16],
                        xwg[:, g : g + 1].to_broadcast([128, 16]),
                        op=OP.mult,
                    )
                    gt = gpool.tile([128, D], BF16, tag="gt")
                    nc.gpsimd.indirect_dma_start(
                        out=gt[:],
                        out_offset=None,
                        in_=cbf[:],
                        in_offset=bass.IndirectOffsetOnAxis(
                            ap=git_all[:, ch * GR + g : ch * GR + g + 1], axis=0
                        ),
                    )
                    for dc in range(8):
                        nc.tensor.matmul(
                            pe_t[:, dc, g * 16 : (g + 1) * 16],
                            gt[:, dc * 128 : (dc + 1) * 128],
                            xwdg[:],
                            start=True,
                            stop=True,
                        )
                embs = opool.tile([128, 8, 128], BF16, tag="embs")
                nc.vector.tensor_copy(embs[:], pe_t[:])
                nc.sync.dma_start(
                    embT.rearrange("(dc p) t -> p dc t", p=128)[:, :, tsl], embs[:]
                )

        # epilogue: all Ln together (single ACT table switch)
        for ch in range(CH):
            lS = small.tile([128, NK], F32, tag="lS")
            lG = small.tile([128, NK], F32, tag="lG")
            nc.scalar.activation(lS[:], S_list[ch][:], AF.Ln)
            nc.scalar.activation(lG[:], G_list[ch][:], AF.Ln)
            xlt = nobuf.tile([128, NK], F32, tag="xlt")
            nc.vector.tensor_tensor(xlt[:], lS[:], lG[:], op=OP.subtract)
            nc.gpsimd.dma_start(xl[ch * 128 : (ch + 1) * 128, :], xlt[:])

    nc.finalize()
    return nc


def _prep_inputs(x, target, codebook, proj_w, proj_b, wproj_w):
    """Host-side prep: shard over batch, transpose/cast for the kernel."""
    B, T, _ = x.shape
    CH = T // 128
    assert not np.any(np.asarray(proj_b)), "kernel assumes proj_b == 0"

    bf = ml_dtypes.bfloat16
    wT = np.ascontiguousarray(np.asarray(proj_w, dtype=np.float32).T).astype(bf)
    wpT = np.ascontiguousarray(np.asarray(wproj_w, dtype=np.float32).T).astype(bf)
    cbf = np.asarray(codebook, dtype=np.float32).reshape(NG, D).astype(bf)
    m32 = np.zeros((128, 32), dtype=bf)
    m32[np.arange(128), np.arange(128) // 8] = 1

    in_maps = []
    for b in range(B):
        xb = np.asarray(x[b], dtype=np.float32)
        tb = np.asarray(target[b], dtype=np.int64)
        gi = (tb * NK + np.arange(NK)[None, :]).astype(np.int32)
        # (T,8) -> (T//16, 128) row-major == (group, (m,k)) -> transpose
        gi = np.ascontiguousarray(gi.reshape(T // 16, 128).T)
        tgf = np.ascontiguousarray(
            tb.astype(np.float32).reshape(CH, 128, NK).transpose(1, 0, 2)
        )
        in_maps.append(
            {
                "xT": np.ascontiguousarray(xb.T).astype(bf),
                "wT": wT,
                "wpT": wpT,
                "cbf": cbf,
                "gidx": gi,
                "tgtf": tgf,
                "m32": m32,
            }
        )
    return in_maps


def kernel(x, target, codebook, proj_w, proj_b, wproj_w):
    from concourse import bass_utils

    x = np.asarray(x)
    B, T, _ = x.shape
    in_maps = _prep_inputs(x, target, codebook, proj_w, proj_b, wproj_w)
    nc = build_bass(T=T)
    res = bass_utils.run_bass_kernel_spmd(nc, in_maps, core_ids=list(range(B)))
    emb = np.stack(
        [res.results[b]["embT"].astype(np.float32).T for b in range(B)], axis=0
    )
    x_loss = np.stack([res.results[b]["xl"] for b in range(B)], axis=0)
    return emb, x_loss


# revision 23
# speedup vs baseline: 1.0770x; 1.0770x over previous
"""Trainium2 Bass kernel for the vq_codebook problem.

Math (per batch row b, all computed on one NeuronCore; batch is data-parallel
across the 8 cores):
  xp[t, n]   = x[t] . proj_w[n]  (+ proj_b[n])          n = k*1024 + c
  x_loss[t,k]= logsumexp_c(xp[t,k,:]) - xp[t,k,tgt]     (no max-subtraction
                needed: |logits| <~ 7 so exp() is safe in fp32/fp16)
  xw[t,:]    = softmax_k(x[t] . wproj_w[k])
  emb[t,:]   = sum_k xw[t,k] * codebook[tgt[t,k], k, :]

Device layout choices:
  - tokens on partitions, codes on the free dim -> the softmax denominator
    falls out of the ScalarE exp pass via accum_out, and the CE "gather"
    is a one-hot tensor_scalar mask + tensor_tensor_reduce on VectorE.
  - codebook rows fetched with indirect DMA (gather) in (token,k)-pair
    partition order; the weighted sum over k is a tiny block-diagonal
    matmul producing emb transposed (d on partitions), which keeps the
    PSUM->SBUF copy at full 128-partition utilization.

Walrus wait-budget notes: several instruction encodings (TensorScalarPtr,
PSEUDO_DMA_DIRECT2D) accept only one sync-wait command, so small per-chunk
tensors are loaded once as resident constants, per-chunk DRAM bounce tiles
get bufs=CH (no slot reuse -> no WAR/WAW waits), and AP-scalar operands are
either produced on VectorE itself or replaced by broadcast tensor_tensor.
"""

import os
import sys

import numpy as np

if "/opt/trn_rl_repo" not in sys.path:
    sys.path.insert(0, "/opt/trn_rl_repo")

import ml_dtypes
from contextlib import ExitStack

import concourse.bass as bass
import concourse.tile as tile
from concourse import bacc
from concourse import mybir

F32 = mybir.dt.float32
BF16 = mybir.dt.bfloat16
F16 = mybir.dt.float16
I32 = mybir.dt.int32

D = 1024          # model dim
NK = 8            # num codebooks
NCODE = 1024      # codes per codebook
NG = NK * NCODE   # 8192 flat codes
N_CORES = 8

AF = mybir.ActivationFunctionType
OP = mybir.AluOpType


def build_bass(T: int = 1024):
    """Build the single-core Bass program (SPMD across cores)."""
    CH = T // 128          # token chunks of 128
    GR = 8                 # groups of 16 tokens per chunk
    nc = bacc.Bacc("TRN2", target_bir_lowering=False)

    xT = nc.dram_tensor("xT", [D, T], BF16, kind="ExternalInput")
    wT = nc.dram_tensor("wT", [128, NK, 8, NCODE], BF16, kind="ExternalInput")
    wpT = nc.dram_tensor("wpT", [D, NK], BF16, kind="ExternalInput")
    cbf = nc.dram_tensor("cbf", [NG, D], BF16, kind="ExternalInput")
    gidx = nc.dram_tensor("gidx", [128, T // 16], I32, kind="ExternalInput")
    tgtf = nc.dram_tensor("tgtf", [128, CH, NK], F32, kind="ExternalInput")
    m32 = nc.dram_tensor("m32", [128, 32], BF16, kind="ExternalInput")
    embT = nc.dram_tensor("embT", [D, T], BF16, kind="ExternalOutput")
    xl = nc.dram_tensor("xl", [T, NK], F32, kind="ExternalOutput")

    with tile.TileContext(nc) as tc, ExitStack() as ctx:
        const = ctx.enter_context(tc.tile_pool(name="const", bufs=1))
        small = ctx.enter_context(tc.tile_pool(name="small", bufs=2))
        epool = ctx.enter_context(tc.tile_pool(name="epool", bufs=2))
        spool = ctx.enter_context(tc.tile_pool(name="spool", bufs=2))
        gpool = ctx.enter_context(tc.tile_pool(name="gpool", bufs=3))
        opool = ctx.enter_context(tc.tile_pool(name="opool", bufs=2))
        nobuf = ctx.enter_context(tc.tile_pool(name="nobuf", bufs=CH))
        drm = ctx.enter_context(tc.tile_pool(name="drm", bufs=CH, space="DRAM"))
        psl = ctx.enter_context(tc.tile_pool(name="psl", bufs=3, space="PSUM"))
        pse = ctx.enter_context(tc.tile_pool(name="pse", bufs=1, space="PSUM"))

        # resident inputs. Urgent/small tensors go on the SWDGE path (async
        # descriptor generation, shared with gathers); x and the 16MB weight
        # stream own the serial HWDGE ring, sliced per-codebook so each slab
        # arrives just-in-time for its k-pass without blocking anyone.
        tg_al0 = const.tile([128, CH, NK], F32)
        nc.gpsimd.dma_start(tg_al0[:], tgtf[:])
        tg_all = const.tile([128, CH, NK], F32)
        nc.vector.tensor_copy(tg_all[:], tg_al0[:])
        git_all = const.tile([128, T // 16], I32)
        nc.gpsimd.dma_start(git_all[:], gidx[:])
        m32_t = const.tile([128, 32], BF16)
        nc.gpsimd.dma_start(m32_t[:], m32[:])
        m32c = const.tile([128, 32], BF16)
        nc.vector.tensor_copy(m32c[:], m32_t[:])
        wp_t = const.tile([128, 8, NK], BF16)
        nc.gpsimd.dma_start(wp_t[:], wpT.rearrange("(dc p) k -> p dc k", p=128))
        x_t = const.tile([128, 8, T], BF16)
        xTr = xT.rearrange("(dc p) t -> p dc t", p=128)
        for dc in range(8):
            nc.sync.dma_start(x_t[:, dc, :], xTr[:, dc, :])
        w_t = const.tile([128, NK, 8, NCODE], BF16)
        for k in range(NK):
            nc.sync.dma_start(w_t[:, k, :, :], wT[:, k, :, :])
        # iota 0..NCODE-1 per partition; the fp16 copy is DVE-produced so the
        # mask TensorScalarPtr carries no extra semaphore waits.
        iota_i = const.tile([128, NCODE], mybir.dt.int16)
        nc.gpsimd.iota(iota_i[:], pattern=[[1, NCODE]], base=0, channel_multiplier=0)
        iota_t = const.tile([128, NCODE], F16)
        nc.vector.tensor_copy(iota_t[:], iota_i[:])

        xwg_list = []
        S_list = [nobuf.tile([128, NK], F32, tag=f"S{c}", name=f"S{c}") for c in range(CH)]
        G_list = [nobuf.tile([128, NK], F32, tag=f"G{c}", name=f"G{c}") for c in range(CH)]

        for k in range(NK):
            for ch in range(CH):
                tsl = slice(ch * 128, (ch + 1) * 128)
                ps = psl.tile([128, NCODE], F32, tag="lg")
                if k == 0:
                    pw = pse.tile([128, NK], F32, tag="pe")
                for d in range(8):
                    lhs = x_t[:, d, tsl]
                    for nb in range(2):
                        nc.tensor.matmul(
                            ps[:, nb * 512 : (nb + 1) * 512],
                            lhs,
                            w_t[:, k, d, nb * 512 : (nb + 1) * 512],
                            start=(d == 0),
                            stop=(d == 7),
                        )
                    if k == 0:
                        nc.tensor.matmul(
                            pw[:], lhs, wp_t[:, d, :], start=(d == 0), stop=(d == 7)
                        )
                E = epool.tile([128, NCODE], F16, tag="E")
                nc.scalar.activation(
                    E[:], ps[:], AF.Exp, accum_out=S_list[ch][:, k : k + 1]
                )
                # G[t] = sum_c (iota_c == tgt_t) * E[t,c] = E[t, tgt_t]
                scrap = spool.tile([128, NCODE], F16, tag="scrap")
                nc.vector.scalar_tensor_tensor(
                    out=scrap[:],
                    in0=iota_t[:],
                    scalar=tg_all[:, ch, k : k + 1],
                    in1=E[:],
                    op0=OP.is_equal,
                    op1=OP.mult,
                    accum_out=G_list[ch][:, k : k + 1],
                )

                # xw softmax for this chunk (needs pw, k==0 only)
                if k == 0:
                    Ew = small.tile([128, NK], F32, tag="Ew")
                    Sw = small.tile([128, 1], F32, tag="Sw")
                    nc.scalar.activation(Ew[:], pw[:], AF.Exp, accum_out=Sw[:])
                    Rw = small.tile([128, 1], F32, tag="Rw")
                    nc.vector.reciprocal(Rw[:], Sw[:])
                    xwb = nobuf.tile([128, NK], BF16, tag="xwb")
                    nc.vector.tensor_tensor(
                        xwb[:], Ew[:], Rw[:, 0:1].to_broadcast([128, NK]), op=OP.mult
                    )
                    # repack xw from (t, k) to partition order p=(t%16)*8+k via
                    # DRAM round-trip: row-major (t,k) == ((g,m),k) == (g,(m,k))
                    xwd_d = drm.tile([128, NK], BF16, tag="xwdram")
                    nc.gpsimd.dma_start(xwd_d[:], xwb[:])
                    xwg = nobuf.tile([128, GR], BF16, tag="xwg", name=f"xwg{ch}")
                    nc.gpsimd.dma_start(
                        xwg[:], xwd_d[:].rearrange("(g m) k -> (m k) g", g=GR)
                    )
                    xwg_list.append(xwg)

                # emb for chunk ch, spread across k-iterations 1..7 so the
                # gather DMAs don't pile up behind one k-pass
                if k != 1 + (ch % 7):
                    continue
                xwg = xwg_list[ch]
                pe_t = pse.tile([128, 8, 128], F32, tag="pe")
                for g in range(GR):
                    xwdg = small.tile([128, 16], BF16, tag="xwdg")
                    nc.vector.tensor_tensor(
                        xwdg[:],
                        m32c[:, 0:16],
                        xwg[:, g : g + 1].to_broadcast([128, 16]),
                        op=OP.mult,
                    )
                    gt = gpool.tile([128, D], BF16, tag="gt")
                    nc.gpsimd.indirect_dma_start(
                        out=gt[:],
                        out_offset=None,
                        in_=cbf[:],
                        in_offset=bass.IndirectOffsetOnAxis(
                            ap=git_all[:, ch * GR + g : ch * GR + g + 1], axis=0
                        ),
                    )
                    for dc in range(8):
                        nc.tensor.matmul(
                            pe_t[:, dc, g * 16 : (g + 1) * 16],
                            gt[:, dc * 128 : (dc + 1) * 128],
                            xwdg[:],
                            start=True,
                            stop=True,
                        )
                embs = opool.tile([128, 8, 128], BF16, tag="embs")
                nc.vector.tensor_copy(embs[:], pe_t[:])
                nc.gpsimd.dma_start(
                    embT.rearrange("(dc p) t -> p dc t", p=128)[:, :, tsl], embs[:]
                )

        # epilogue: all Ln together (single ACT table switch)
        for ch in range(CH):
            lS = small.tile([128, NK], F32, tag="lS")
            lG = small.tile([128, NK], F32, tag="lG")
            nc.scalar.activation(lS[:], S_list[ch][:], AF.Ln)
            nc.scalar.activation(lG[:], G_list[ch][:], AF.Ln)
            xlt = nobuf.tile([128, NK], F32, tag="xlt")
            nc.vector.tensor_tensor(xlt[:], lS[:], lG[:], op=OP.subtract)
            nc.gpsimd.dma_start(xl[ch * 128 : (ch + 1) * 128, :], xlt[:])

    nc.finalize()
    return nc


def _prep_inputs(x, target, codebook, proj_w, proj_b, wproj_w):
    """Host-side prep: shard over batch, transpose/cast for the kernel."""
    B, T, _ = x.shape
    CH = T // 128
    assert not np.any(np.asarray(proj_b)), "kernel assumes proj_b == 0"

    bf = ml_dtypes.bfloat16
    wT = np.ascontiguousarray(
        np.asarray(proj_w, dtype=np.float32)
        .reshape(NK, NCODE, 8, 128)
        .transpose(3, 0, 2, 1)
    ).astype(bf)
    wpT = np.ascontiguousarray(np.asarray(wproj_w, dtype=np.float32).T).astype(bf)
    cbf = np.asarray(codebook, dtype=np.float32).reshape(NG, D).astype(bf)
    m32 = np.zeros((128, 32), dtype=bf)
    m32[np.arange(128), np.arange(128) // 8] = 1

    in_maps = []
    for b in range(B):
        xb = np.asarray(x[b], dtype=np.float32)
        tb = np.asarray(target[b], dtype=np.int64)
        gi = (tb * NK + np.arange(NK)[None, :]).astype(np.int32)
        # (T,8) -> (T//16, 128) row-major == (group, (m,k)) -> transpose
        gi = np.ascontiguousarray(gi.reshape(T // 16, 128).T)
        tgf = np.ascontiguousarray(
            tb.astype(np.float32).reshape(CH, 128, NK).transpose(1, 0, 2)
        )
        in_maps.append(
            {
                "xT": np.ascontiguousarray(xb.T).astype(bf),
                "wT": wT,
                "wpT": wpT,
                "cbf": cbf,
                "gidx": gi,
                "tgtf": tgf,
                "m32": m32,
            }
        )
    return in_maps


def kernel(x, target, codebook, proj_w, proj_b, wproj_w):
    from concourse import bass_utils

    x = np.asarray(x)
    B, T, _ = x.shape
    in_maps = _prep_inputs(x, target, codebook, proj_w, proj_b, wproj_w)
    nc = build_bass(T=T)
    res = bass_utils.run_bass_kernel_spmd(nc, in_maps, core_ids=list(range(B)))
    emb = np.stack(
        [res.results[b]["embT"].astype(np.float32).T for b in range(B)], axis=0
    )
    x_loss = np.stack([res.results[b]["xl"] for b in range(B)], axis=0)
    return emb, x_loss
